# revision 6
# baseline (speedup 1.0000x reference)
"""CosineSimilarityLoss (histogram binning) Trainium2 kernel, v2.

Full inputs [2048, 4096] f32 x5 -> scalar f32 loss = 1 - mean(cosine_sim).

Data-parallel over 8 cores (256 rows each). Per core the per-row
2000-bin histogram is built as a 64x32 two-level histogram via the
tensor engine:

  bin = floor(y), y = min(mz*2000, 1999.5)   (clip edge handled by min)
  a = bin >> 5 (coarse, 64 values as k32 = bin - bin%32)
  b = bin % 32 (fine)
  H[a, b] = sum_k w_k * [k32_k == 32a] * [bmod_k == b]
          = U^T V  with U[k,a] = [k32_k == 32a], V[k,b] = w_k*[bmod_k == b]

Items are moved to the partition (contraction) axis with PE
transposes; U/V one-hots are built by DVE broadcast-compare in fp16
(exact: all compared values are small integers); 32 rows per matmul
batch accumulate over 32 item-chunks into PSUM. dot/pn2/tn2 come from
elementwise psum products reduced by a ones-matmul + segmented
tensor_reduce; the cosine tail matches the reference's eps semantics.
"""

import os
import sys

sys.path.insert(0, "/opt/trn_rl_repo")

import numpy as np

import concourse.bass as bass
from concourse import bacc, mybir
from concourse.bass_utils import run_bass_kernel_spmd
from concourse.tile import TileContext
from concourse.masks import make_identity

N_CORES = 8
B, P, T = 2048, 4096, 4096
ROWS_PER_CORE = B // N_CORES  # 256
NA = 64   # coarse bins (values 32*a)
NB = 32   # fine bins
EPS = 1e-8
F32 = mybir.dt.float32
F16 = mybir.dt.float16
I32 = mybir.dt.int32
ALU = mybir.AluOpType
AXL = mybir.AxisListType
ACT = mybir.ActivationFunctionType

HALF = os.environ.get("KERNEL_HALF", "0") == "1"
NT = 2          # row-tiles per core (128 rows each)
NQ = 4          # quarters per row-tile (32 rows)
QR = 32         # rows per quarter
NCHUNK = 32     # item chunks of 128

_NC_CACHE = {}


def build_nc():
    if "nc" in _NC_CACHE:
        return _NC_CACHE["nc"]
    nc = bacc.Bacc("TRN2", target_bir_lowering=False, debug=False, num_devices=N_CORES)
    d_pmz = nc.dram_tensor("pmz", [ROWS_PER_CORE, P], F32, kind="ExternalInput")
    d_pint = nc.dram_tensor("pint", [ROWS_PER_CORE, P], F32, kind="ExternalInput")
    d_tmz = nc.dram_tensor("tmz", [ROWS_PER_CORE, T], F32, kind="ExternalInput")
    d_tint = nc.dram_tensor("tint", [ROWS_PER_CORE, T], F32, kind="ExternalInput")
    d_tmask = nc.dram_tensor("tmask", [ROWS_PER_CORE, T], F32, kind="ExternalInput")
    d_cos = nc.dram_tensor("cos", [1, ROWS_PER_CORE], F32, kind="ExternalOutput")

    with TileContext(nc) as tc:
        with (
            tc.tile_pool(name="const", bufs=1) as cp,
            tc.tile_pool(name="raw", bufs=1) as raw,
            tc.tile_pool(name="pre", bufs=1) as pre,
            tc.tile_pool(name="tx", bufs=1) as tx,
            tc.tile_pool(name="uv", bufs=2) as uv,
            tc.tile_pool(name="xx", bufs=2) as xx,
            tc.tile_pool(name="red", bufs=1) as red,
            tc.tile_pool(name="tp", bufs=2, space="PSUM") as tp,
            tc.tile_pool(name="hp", bufs=1, space="PSUM") as hpp,
            tc.tile_pool(name="ht", bufs=1, space="PSUM") as htp,
            tc.tile_pool(name="sp", bufs=1, space="PSUM") as sp,
        ):
            # ---- constants ----
            ident = cp.tile([128, 128], F16, tag="ident")
            make_identity(nc, ident[:])
            ca_i = cp.tile([128, NA], I32, tag="ca_i")
            nc.gpsimd.iota(ca_i[:], pattern=[[32, NA]], base=0, channel_multiplier=0)
            const_a = cp.tile([128, NA], F16, tag="ca")
            nc.vector.tensor_copy(const_a[:], ca_i[:])
            cb_i = cp.tile([128, NB], I32, tag="cb_i")
            nc.gpsimd.iota(cb_i[:], pattern=[[1, NB]], base=0, channel_multiplier=0)
            const_b = cp.tile([128, NB], F16, tag="cb")
            nc.vector.tensor_copy(const_b[:], cb_i[:])
            ones64 = cp.tile([NA, 1], F16, tag="ones")
            nc.gpsimd.memset(ones64[:], 1.0)

            # ---- collectors ----
            dotc = red.tile([1, ROWS_PER_CORE], F32, tag="dotc")
            pnc = red.tile([1, ROWS_PER_CORE], F32, tag="pnc")
            tnc = red.tile([1, ROWS_PER_CORE], F32, tag="tnc")

            for t in range(NT):
                rs = slice(128 * t, 128 * (t + 1))
                txT = []  # per spectrum: (k32T, bmT, wT)
                for s in range(2):
                    mz = raw.tile([128, P], F32, tag="mz")
                    w = raw.tile([128, P], F32, tag="w")
                    scr = raw.tile([128, P], F32, tag="scr")
                    wh = pre.tile([128, P], F16, tag=f"wh{s}")
                    if s == 0:
                        nc.sync.dma_start(mz[:], d_pmz[rs, :])
                        nc.sync.dma_start(w[:], d_pint[rs, :])
                        nc.scalar.copy(wh[:], w[:])
                    else:
                        nc.sync.dma_start(mz[:], d_tmz[rs, :])
                        nc.sync.dma_start(w[:], d_tint[rs, :])
                        nc.sync.dma_start(scr[:], d_tmask[rs, :])
                        nc.gpsimd.tensor_mul(wh[:], w[:], scr[:])
                    # y = min(mz*2000, 1999.5); ki = int(y) (trunc);
                    # bmod = ki & 31; k32 = ki & ~31
                    # (f32->i32 convert is round-to-nearest on DVE, so
                    # subtract 0.5 first to get floor semantics)
                    nc.vector.tensor_scalar(
                        mz[:], mz[:], 2000.0, 1999.5, ALU.mult, ALU.min
                    )
                    ki = w[:].bitcast(I32)
                    bi = scr[:].bitcast(I32)
                    nc.vector.tensor_single_scalar(ki, mz[:], 0.5, ALU.subtract)
                    k32h = pre.tile([128, P], F16, tag=f"k32h{s}")
                    bmodh = pre.tile([128, P], F16, tag=f"bmodh{s}")
                    nc.vector.tensor_single_scalar(bi, ki, 31, ALU.bitwise_and)
                    nc.vector.tensor_copy(bmodh[:], bi)
                    nc.vector.tensor_single_scalar(bi, ki, -32, ALU.bitwise_and)
                    nc.vector.tensor_copy(k32h[:], bi)

                    # transposes: items -> partitions, groups of 4 blocks
                    k32T = tx.tile([128, P], F16, tag=f"k32T{s}")
                    bmT = tx.tile([128, P], F16, tag=f"bmT{s}")
                    wT = tx.tile([128, P], F16, tag=f"wT{s}")
                    for src, dst in ((k32h, k32T), (bmodh, bmT), (wh, wT)):
                        for g in range(8):
                            ps = tp.tile([128, 512], F16, tag="tps")
                            for i in range(4):
                                blk = 4 * g + i
                                nc.tensor.transpose(
                                    ps[:, 128 * i : 128 * (i + 1)],
                                    src[:, 128 * blk : 128 * (blk + 1)],
                                    ident[:],
                                )
                            nc.scalar.copy(dst[:, 512 * g : 512 * (g + 1)], ps[:])
                    txT.append((k32T, bmT, wT))

                for q in range(NQ):
                    hP = hpp.tile([NA, QR * NB], F32, tag="hP")
                    hT = htp.tile([NA, QR * NB], F32, tag="hT")
                    hps = (hP, hT)
                    for c in range(NCHUNK):
                        base = 128 * c + QR * q
                        for s in range(2):
                            k32T, bmT, wT = txT[s]
                            U = uv.tile([128, QR, NA], F16, tag=f"u{s}")
                            vb = uv.tile([128, QR, NB], F16, tag="vb")
                            V = uv.tile([128, QR, NB], F16, tag=f"v{s}")
                            nc.vector.tensor_tensor(
                                U[:],
                                k32T[:, base : base + QR].unsqueeze(2).to_broadcast(
                                    [128, QR, NA]
                                ),
                                const_a[:].unsqueeze(1).to_broadcast([128, QR, NA]),
                                ALU.is_equal,
                            )
                            nc.vector.tensor_tensor(
                                vb[:],
                                bmT[:, base : base + QR].unsqueeze(2).to_broadcast(
                                    [128, QR, NB]
                                ),
                                const_b[:].unsqueeze(1).to_broadcast([128, QR, NB]),
                                ALU.is_equal,
                            )
                            nc.vector.tensor_tensor(
                                V[:],
                                vb[:],
                                wT[:, base : base + QR].unsqueeze(2).to_broadcast(
                                    [128, QR, NB]
                                ),
                                ALU.mult,
                            )
                            h = hps[s]
                            for rm in range(QR):
                                nc.tensor.matmul(
                                    h[:, NB * rm : NB * (rm + 1)],
                                    lhsT=U[:, rm, :],
                                    rhs=V[:, rm, :],
                                    start=(c == 0),
                                    stop=(c == NCHUNK - 1),
                                    skip_group_check=True,
                                )
                    # ---- reduce quarter: dot, pn2, tn2 for 32 rows ----
                    # (DVE has one PSUM read port: evacuate hT first)
                    he = xx.tile([NA, QR * NB], F16, tag="he")
                    xd = xx.tile([NA, QR * NB], F16, tag="xd")
                    xp = xx.tile([NA, QR * NB], F16, tag="xp")
                    xt = xx.tile([NA, QR * NB], F16, tag="xt")
                    nc.scalar.copy(he[:], hT[:])
                    nc.vector.tensor_mul(xd[:], hP[:], he[:])
                    nc.scalar.activation(xp[:], hP[:], ACT.Square)
                    nc.scalar.activation(xt[:], he[:], ACT.Square)
                    ro = 128 * t + QR * q
                    for x, coll in ((xd, dotc), (xp, pnc), (xt, tnc)):
                        s3 = sp.tile([1, QR, NB], F32, tag="s3")
                        half = QR * NB // 2
                        nc.tensor.matmul(
                            s3[:, : QR // 2, :],
                            lhsT=ones64[:],
                            rhs=x[:, :half],
                            start=True,
                            stop=True,
                            skip_group_check=True,
                        )
                        nc.tensor.matmul(
                            s3[:, QR // 2 :, :],
                            lhsT=ones64[:],
                            rhs=x[:, half:],
                            start=True,
                            stop=True,
                            skip_group_check=True,
                        )
                        nc.vector.tensor_reduce(
                            coll[:, ro : ro + QR], s3[:], AXL.X, ALU.add
                        )

            # ---- cosine tail on [1, 256] ----
            pn = red.tile([1, ROWS_PER_CORE], F32, tag="pn")
            tn = red.tile([1, ROWS_PER_CORE], F32, tag="tn")
            rp = red.tile([1, ROWS_PER_CORE], F32, tag="rp")
            rt = red.tile([1, ROWS_PER_CORE], F32, tag="rt")
            den = red.tile([1, ROWS_PER_CORE], F32, tag="den")
            cosv = red.tile([1, ROWS_PER_CORE], F32, tag="cosv")
            nc.scalar.activation(pn[:], pnc[:], ACT.Sqrt)
            nc.scalar.activation(tn[:], tnc[:], ACT.Sqrt)
            nc.vector.tensor_scalar_add(rp[:], pn[:], EPS)
            nc.vector.reciprocal(rp[:], rp[:])
            nc.vector.tensor_scalar_add(rt[:], tn[:], EPS)
            nc.vector.reciprocal(rt[:], rt[:])
            nc.vector.tensor_mul(dotc[:], dotc[:], rp[:])
            nc.vector.tensor_mul(dotc[:], dotc[:], rt[:])
            nc.vector.tensor_mul(pn[:], pn[:], rp[:])
            nc.vector.tensor_mul(tn[:], tn[:], rt[:])
            nc.vector.tensor_scalar_max(pn[:], pn[:], EPS)
            nc.vector.tensor_scalar_max(tn[:], tn[:], EPS)
            nc.vector.tensor_mul(den[:], pn[:], tn[:])
            nc.vector.reciprocal(den[:], den[:])
            nc.vector.tensor_mul(cosv[:], dotc[:], den[:])
            nc.sync.dma_start(d_cos[:], cosv[:])
    nc.compile()
    _NC_CACHE["nc"] = nc
    return nc


def make_in_maps(np_inputs):
    in_maps = []
    for c in range(N_CORES):
        rs = slice(c * ROWS_PER_CORE, (c + 1) * ROWS_PER_CORE)
        in_maps.append(
            {
                "pmz": np.ascontiguousarray(np_inputs["pred_mz"][rs]),
                "pint": np.ascontiguousarray(np_inputs["pred_intensity"][rs]),
                "tmz": np.ascontiguousarray(np_inputs["target_mz"][rs]),
                "tint": np.ascontiguousarray(np_inputs["target_intensity"][rs]),
                "tmask": np.ascontiguousarray(np_inputs["target_mask"][rs]),
            }
        )
    return in_maps


def kernel(pred_mz, pred_intensity, target_mz, target_intensity, target_mask):
    pred_mz = np.ascontiguousarray(pred_mz, dtype=np.float32)
    pred_intensity = np.ascontiguousarray(pred_intensity, dtype=np.float32)
    target_mz = np.ascontiguousarray(target_mz, dtype=np.float32)
    target_intensity = np.ascontiguousarray(target_intensity, dtype=np.float32)
    target_mask = np.ascontiguousarray(target_mask, dtype=np.float32)

    nc = build_nc()
    in_maps = make_in_maps(
        {
            "pred_mz": pred_mz,
            "pred_intensity": pred_intensity,
            "target_mz": target_mz,
            "target_intensity": target_intensity,
            "target_mask": target_mask,
        }
    )
    res = run_bass_kernel_spmd(nc, in_maps, core_ids=list(range(N_CORES)))
    cos = np.concatenate([r["cos"].reshape(-1) for r in res.results])
    mean = np.mean(cos.astype(np.float64))
    return np.float32(1.0 - mean)
